# revision 18
# baseline (speedup 1.0000x reference)
"""Dense Synthesizer Attention — Trainium2 Bass kernel (v3).

Sharding: data-parallel over batch. B=8 batch elements, 8 NeuronCores,
one batch element per core, zero collectives.

Per-core (S=1024, F=512, H=8 heads, dk=64), bf16 matmuls, fp32 PSUM:
    hT  = relu(w1^T @ qT + b1)          [1024, 1024]
    awT = w2^T @ hT + b2                [512, 1024]
    per head pair: scores via ROW-TILED concurrent A/B matmuls
    (explicit tile_position (0,0)/(64,0)), E = exp(scores/8) on ScalarE,
    rowsums on DVE tensor_reduce (bf16 2x), attn_v via COL-TILED
    concurrent A/B matmuls, rbc = broadcast 1/rowsum via PE transposes,
    yT evac multiplies by rbc, then single-pass out projection.

Schedule: the exp stream on ScalarE is the pacer (64 x ~1.15us) and must
run back-to-back. Same-tiling-mode matmuls are grouped into bursts (mode
switches cost ~100ns). The scores PSUM ring is 2 deep so semaphore waits
resolve ahead of the PE (v2's single-buffered psA/psB serialized the A/B
pair at ACT cadence). PSUM banks: scores ring 2x[128,1024] (4) + pinned
[128,1024] (2: mlp2 chunks, then attn_v(3) accumulation; rbc transposes
go through the ring) + 2x[128,512] (vproj/attn_v/outproj).
"""

import math

import numpy as np

B, S, F = 8, 1024, 512
H, DK = 8, 64
HID = 2 * F
P = 128

N_CORES = 8

# pairs whose rowsums come from ACT accum_out instead of DVE tensor_reduce
ACT_ROWSUM_PAIRS = (3,)

_CACHED_NC = None


def _build_nc():
    from contextlib import ExitStack

    import concourse.mybir as mybir
    import concourse.tile as tile
    from concourse import bacc

    dt = mybir.dt
    f32 = dt.float32
    bf16 = dt.bfloat16
    AF = mybir.ActivationFunctionType
    ALU = mybir.AluOpType
    AX = mybir.AxisListType

    SC = S // P      # 8 token chunks
    FC = F // P      # 4 feature chunks
    KC = HID // P    # 8 hidden chunks
    NS = S // 512    # 2 moving-dim chunks
    NH = H // 2      # 4 head pairs

    nc = bacc.Bacc(
        "TRN2",
        target_bir_lowering=False,
        debug=False,
        num_devices=N_CORES,
    )

    q_d = nc.declare_dram_parameter("qT", [F, S], bf16, isOutput=False)
    v_d = nc.declare_dram_parameter("vT", [F, S], bf16, isOutput=False)
    w1_d = nc.declare_dram_parameter("w1", [F, HID], bf16, isOutput=False)
    w2_d = nc.declare_dram_parameter("w2", [HID, F], bf16, isOutput=False)
    wv_d = nc.declare_dram_parameter("wv", [F, F], bf16, isOutput=False)
    wo_d = nc.declare_dram_parameter("wo", [F, F], bf16, isOutput=False)
    b1_d = nc.declare_dram_parameter("b1r", [P, KC], f32, isOutput=False)
    b2_d = nc.declare_dram_parameter("b2r", [P, FC], f32, isOutput=False)
    bo_d = nc.declare_dram_parameter("bo2b", [P, F], f32, isOutput=False)
    id_d = nc.declare_dram_parameter("ident", [P, P], f32, isOutput=False)
    out_d = nc.declare_dram_parameter("out", [S, F], f32, isOutput=True)

    scale = 1.0 / math.sqrt(DK)

    with ExitStack() as ctx:
        tc = ctx.enter_context(tile.TileContext(nc))

        const = ctx.enter_context(tc.tile_pool(name="const", bufs=1))
        big = ctx.enter_context(tc.tile_pool(name="big", bufs=1))
        epool = ctx.enter_context(tc.tile_pool(name="ep", bufs=4))
        rpool = ctx.enter_context(tc.tile_pool(name="rp", bufs=2))
        opool = ctx.enter_context(tc.tile_pool(name="op", bufs=4))
        rbcpool = ctx.enter_context(tc.tile_pool(name="rbc", bufs=2))
        reppool = ctx.enter_context(tc.tile_pool(name="rep", bufs=2))

        ps_ring = ctx.enter_context(tc.tile_pool(name="psr", bufs=2, space="PSUM"))
        ps_pin = ctx.enter_context(tc.tile_pool(name="psp", bufs=1, space="PSUM"))
        ps_sml = ctx.enter_context(tc.tile_pool(name="pss", bufs=2, space="PSUM"))

        # ---- constants ----
        w1r = w1_d.rearrange("(c p) k -> p c k", p=P)
        w1sb = []
        w1_eng = [nc.scalar, nc.scalar, nc.gpsimd, None]
        for c in range(FC):
            t = const.tile([P, HID], bf16, name=f"w1c{c}")
            if w1_eng[c] is not None:
                w1_eng[c].dma_start(t, w1r[:, c, :])
            w1sb.append(t)
        b1sb = const.tile([P, KC], f32)
        nc.gpsimd.dma_start(b1sb, b1_d[:, :])
        w2sb = const.tile([P, KC, F], bf16)
        wvsb = const.tile([P, FC, F], bf16)
        wosb = const.tile([P, FC, F], bf16)
        b2sb = const.tile([P, FC], f32)
        bosb = const.tile([P, F], f32)
        identsb = const.tile([P, P], f32)

        def mid_consts():
            nc.gpsimd.dma_start(w2sb, w2_d.rearrange("(c p) f -> p c f", p=P))
            nc.gpsimd.dma_start(b2sb, b2_d[:, :])

        def late_consts():
            nc.gpsimd.dma_start(wvsb, wv_d.rearrange("(c p) f -> p c f", p=P))
            nc.gpsimd.dma_start(identsb, id_d[:, :])

        def last_consts():
            nc.gpsimd.dma_start(wosb, wo_d.rearrange("(c p) f -> p c f", p=P))
            nc.gpsimd.dma_start(bosb, bo_d[:, :])

        # ---- inputs ----
        qTsb = big.tile([P, FC, S], bf16, tag="qx")
        qr = q_d.rearrange("(c p) s -> p c s", p=P)
        nc.sync.dma_start(qTsb[:, :, :512], qr[:, :, :512])
        nc.sync.dma_start(qTsb[:, :, 512:], qr[:, :, 512:])
        nc.sync.dma_start(w1sb[3], w1r[:, 3, :])
        valTsb = big.tile([P, FC, S], bf16, tag="vT")
        nc.sync.dma_start(valTsb, v_d.rearrange("(c p) s -> p c s", p=P))
        mid_consts()

        hTsb = big.tile([P, KC, S], bf16, tag="hT")
        awTsb = big.tile([P, FC, S], bf16, tag="awT")
        vsb = big.tile([P, SC, F], bf16, tag="v")
        yTsb = big.tile([P, FC, S], bf16, tag="qx")  # reuses qT slot

        e_tiles = {}
        rsums = [None] * NH
        rinvs = [None] * NH
        rbcs = [None] * NH

        # ---- HAM warm-up: memset-seeded (no DMA dependency), one
        #      accumulation group (no inter-MM semaphores, gapless) ----
        warm_sb = const.tile([P, 512], bf16, name="warmsb")
        nc.vector.memset(warm_sb, 0.0)
        wt = ps_ring.tile([P, S], f32, tag="ring", name="warm")
        NWARM = 12
        for i in range(NWARM):
            nc.tensor.matmul(wt[:, 0:512], warm_sb[:, 0:P], warm_sb[:, 0:512],
                             start=(i == 0), stop=(i == NWARM - 1))

        # ---- mlp1 (full mode); mlp2(fc0) partials ride along in the
        #      pinned PSUM tile as hT chunks complete ----
        m2pin = {}

        def mlp2_fc0_fill(c):
            if c == 0:
                m2pin[0] = ps_pin.tile([P, S], f32, tag="pin", name="m2f")
            for n in range(NS):
                nc.tensor.matmul(
                    m2pin[0][:, n * 512:(n + 1) * 512],
                    w2sb[:, c, 0:P],
                    hTsb[:, c, n * 512:(n + 1) * 512],
                    start=(c == 0),
                    stop=(c == KC - 1),
                )

        for k in range(KC):
            ps = ps_ring.tile([P, S], f32, tag="ring", name=f"m1k{k}")
            for n in range(NS):
                for c in range(FC):
                    nc.tensor.matmul(
                        ps[:, n * 512:(n + 1) * 512],
                        w1sb[c][:, k * P:(k + 1) * P],
                        qTsb[:, c, n * 512:(n + 1) * 512],
                        start=(c == 0),
                        stop=(c == FC - 1),
                    )
            nc.vector.tensor_scalar(
                hTsb[:, k, :], ps, b1sb[:, k:k + 1], 0.0, ALU.add, ALU.max,
            )
            if k >= 1:
                mlp2_fc0_fill(k - 1)
            if k == 1:
                late_consts()
            if k == 4:
                last_consts()
        mlp2_fc0_fill(KC - 1)
        nc.vector.tensor_scalar_add(awTsb[:, 0, :], m2pin[0], b2sb[:, 0:1])

        # ---- mlp2 chunk fc as two half-bursts (pinned PSUM); each half
        #      fits in the PE slack of one exp slot ----
        _m2pin = {}

        def mlp2_burst(fc, n):
            if n == 0:
                _m2pin[fc] = ps_pin.tile([P, S], f32, tag="pin", name=f"m2p{fc}")
            pin = _m2pin[fc]
            for c in range(KC):
                nc.tensor.matmul(
                    pin[:, n * 512:(n + 1) * 512],
                    w2sb[:, c, fc * P:(fc + 1) * P],
                    hTsb[:, c, n * 512:(n + 1) * 512],
                    start=(c == 0),
                    stop=(c == KC - 1),
                )
            if n == NS - 1:
                nc.vector.tensor_scalar_add(
                    awTsb[:, fc, :], pin, b2sb[:, fc:fc + 1])

        # ---- v projection for token tile m (full mode) ----
        def vproj(m):
            t = ps_sml.tile([P, F], f32, tag="sml", name="vpp")
            for c in range(FC):
                nc.tensor.matmul(
                    t,
                    valTsb[:, c, m * P:(m + 1) * P],
                    wvsb[:, c, :],
                    start=(c == 0),
                    stop=(c == FC - 1),
                )
            nc.vector.tensor_copy(vsb[:, m, :], t)

        # ---- scores slot (row-tiled pair, ring PSUM) + exp + rowsums ----
        def scores_slot(q, m):
            eA, eB = e_tiles[(q, 0)], e_tiles[(q, 1)]
            a_l = awTsb[0:DK, q, m * P:(m + 1) * P]
            b_l = awTsb[DK:P, q, m * P:(m + 1) * P]
            psa = ps_ring.tile([P, S], f32, tag="ring", name="sA")
            psb = ps_ring.tile([P, S], f32, tag="ring", name="sB")
            for n in range(NS):
                nc.tensor.matmul(
                    psa[:, n * 512:(n + 1) * 512],
                    a_l,
                    awTsb[0:DK, q, n * 512:(n + 1) * 512],
                    start=True, stop=True, tile_position=(0, 0),
                )
                nc.tensor.matmul(
                    psb[:, n * 512:(n + 1) * 512],
                    b_l,
                    awTsb[DK:P, q, n * 512:(n + 1) * 512],
                    start=True, stop=True, tile_position=(64, 0),
                )
            if q in ACT_ROWSUM_PAIRS:
                nc.scalar.activation(eA[:, m, :], psa, AF.Exp, scale=scale,
                                     accum_out=rsums[q][:, 0, m:m + 1])
                nc.scalar.activation(eB[:, m, :], psb, AF.Exp, scale=scale,
                                     accum_out=rsums[q][:, 1, m:m + 1])
            else:
                # head A rowsum on ACT, head B on DVE (balances both queues)
                nc.scalar.activation(eA[:, m, :], psa, AF.Exp, scale=scale,
                                     accum_out=rsums[q][:, 0, m:m + 1])
                nc.scalar.activation(eB[:, m, :], psb, AF.Exp, scale=scale)
                nc.vector.tensor_reduce(
                    rsums[q][:, 1, m:m + 1], eB[:, m, :], AX.X, ALU.add)

        # ---- rbc for pair q: recip + rep (DVE), 8 PE transposes into a
        #      ring tile, bounce to bf16 SBUF ----
        def rbc_build(q):
            nc.vector.reciprocal(rinvs[q], rsums[q])
            rep = reppool.tile([P, SC, 2, DK], f32, name="rep")
            nc.vector.tensor_copy(
                rep,
                rinvs[q][:, :, :].rearrange("p h m -> p m h")
                .broadcast_to((P, SC, 2, DK)),
            )
            rp = ps_pin.tile([P, S], f32, tag="pin", name="rbcp")
            for m in range(SC):
                nc.tensor.transpose(
                    rp[:, m * P:(m + 1) * P],
                    rep[:, m, :, :],
                    identsb[:, :],
                )
            rbcs[q] = rbcpool.tile([P, S], bf16, tag="rbc", name="rbcs")
            nc.vector.tensor_copy(rbcs[q], rp)

        # ---- attn_v (col-tiled pair). q<3 accumulates in a small tile per
        #      n-half; q==3 accumulates both halves in the pinned tile. ----
        _av_ps = {}

        def av_mms(q, n, chunks):
            eA, eB = e_tiles[(q, 0)], e_tiles[(q, 1)]
            for c in chunks:
                if c == 0:
                    _av_ps[(q, n)] = ps_sml.tile(
                        [P, F], f32, tag="sml", name="avp")
                t = _av_ps[(q, n)]
                nc.tensor.matmul(
                    t[0:DK, :],
                    vsb[:, c, (2 * q) * DK:(2 * q + 1) * DK],
                    eA[:, c, n * 512:(n + 1) * 512],
                    start=(c == 0), stop=(c == SC - 1), tile_position=(0, 0),
                )
                nc.tensor.matmul(
                    t[DK:P, :],
                    vsb[:, c, (2 * q + 1) * DK:(2 * q + 2) * DK],
                    eB[:, c, n * 512:(n + 1) * 512],
                    start=(c == 0), stop=(c == SC - 1), tile_position=(0, 64),
                )

        def av_fin(q, n):
            nc.vector.tensor_mul(
                yTsb[:, q, n * 512:(n + 1) * 512],
                _av_ps[(q, n)],
                rbcs[q][:, n * 512:(n + 1) * 512],
            )

        # ---- out projection, split: half1 contracts yT chunks 0,1
        #      (pairs 0,1) into o1 early; half2 adds chunks 2,3 + bias ----
        o1_sb = big.tile([P, SC, F], f32, tag="o1")

        def outproj_half1(m):
            t = ps_sml.tile([P, F], f32, tag="sml", name="op1")
            for c in (0, 1):
                nc.tensor.matmul(
                    t,
                    yTsb[:, c, m * P:(m + 1) * P],
                    wosb[:, c, :],
                    start=(c == 0),
                    stop=(c == 1),
                )
            nc.vector.tensor_add(o1_sb[:, m, :], t, bosb)

        def outproj_half2(m):
            t = ps_sml.tile([P, F], f32, tag="sml", name="op2")
            for c in (2, 3):
                nc.tensor.matmul(
                    t,
                    yTsb[:, c, m * P:(m + 1) * P],
                    wosb[:, c, :],
                    start=(c == 2),
                    stop=(c == 3),
                )
            o_sb = opool.tile([P, F], f32, tag="o", name="osb")
            nc.vector.tensor_add(o_sb, t, o1_sb[:, m, :])
            eng = nc.sync if m % 2 == 0 else nc.scalar
            eng.dma_start(out_d[m * P:(m + 1) * P, :], o_sb)

        def outproj_mid2(m):
            # stage yT chunk 2 (pair 2) into o1 during window 3
            t = ps_sml.tile([P, F], f32, tag="sml", name="om2")
            nc.tensor.matmul(t, yTsb[:, 2, m * P:(m + 1) * P], wosb[:, 2, :],
                             start=True, stop=True)
            nc.vector.tensor_add(o1_sb[:, m, :], t, o1_sb[:, m, :])

        def outproj_c3(m):
            t = ps_sml.tile([P, F], f32, tag="sml", name="oc3")
            nc.tensor.matmul(t, yTsb[:, 3, m * P:(m + 1) * P], wosb[:, 3, :],
                             start=True, stop=True)
            o_sb = opool.tile([P, F], f32, tag="o", name="osb")
            nc.vector.tensor_add(o_sb, t, o1_sb[:, m, :])
            eng = nc.sync if m % 2 == 0 else nc.scalar
            eng.dma_start(out_d[m * P:(m + 1) * P, :], o_sb)

        def rbc_build3_split():
            # pair-3 rbc pipelined in halves through the pin tile
            q = 3
            nc.vector.reciprocal(rinvs[q], rsums[q])
            rep = reppool.tile([P, SC, 2, DK], f32, name="rep")
            nc.vector.tensor_copy(
                rep,
                rinvs[q][:, :, :].rearrange("p h m -> p m h")
                .broadcast_to((P, SC, 2, DK)),
            )
            rp = ps_pin.tile([P, S], f32, tag="pin", name="rbcp3")
            rbcs[q] = rbcpool.tile([P, S], bf16, tag="rbc", name="rbcs")
            for m in range(4):
                nc.tensor.transpose(rp[:, m * P:(m + 1) * P], rep[:, m, :, :],
                                    identsb[:, :])
            nc.scalar.copy(rbcs[q][:, 0:512], rp[:, 0:512])
            av_fin(3, 0)
            for m in range(4):
                outproj_c3(m)
            for m in range(4, SC):
                nc.tensor.transpose(rp[:, m * P:(m + 1) * P], rep[:, m, :, :],
                                    identsb[:, :])
            nc.scalar.copy(rbcs[q][:, 512:], rp[:, 512:])
            av_fin(3, 1)
            for m in range(4, SC):
                outproj_c3(m)

        def alloc_pair(q):
            e_tiles[(q, 0)] = epool.tile([P, SC, S], bf16, tag="e", name="eA")
            e_tiles[(q, 1)] = epool.tile([P, SC, S], bf16, tag="e", name="eB")
            rsums[q] = rpool.tile([P, 2, SC], f32, tag="rs", name="rs")
            rinvs[q] = rpool.tile([P, 2, SC], f32, tag="ri", name="ri")

        # =========== pipeline ===========
        # window 0: scores(0) | mlp2 fc1 | vproj
        alloc_pair(0)
        for m in range(SC):
            scores_slot(0, m)
            if m == 1:
                mlp2_burst(1, 0)
            if m == 2:
                mlp2_burst(1, 1)
            if m >= 4:
                vproj(2 * (m - 4))
                vproj(2 * (m - 4) + 1)
        # window 1: scores(1) | attn_v(0) | mlp2 fc2 | rbc(0)
        alloc_pair(1)
        for m in range(SC):
            scores_slot(1, m)
            if m == 1:
                mlp2_burst(2, 0)
            if m == 2:
                mlp2_burst(2, 1)
            if m == 3:
                av_mms(0, 0, range(SC))
            if m == 5:
                rbc_build(0)
                av_fin(0, 0)
            if m == 6:
                av_mms(0, 1, range(SC))
        av_fin(0, 1)
        # window 2: scores(2) | attn_v(1) | mlp2 fc3 | rbc(1)
        alloc_pair(2)
        for m in range(SC):
            scores_slot(2, m)
            if m == 1:
                mlp2_burst(3, 0)
            if m == 2:
                mlp2_burst(3, 1)
            if m == 3:
                av_mms(1, 0, range(SC))
            if m == 5:
                rbc_build(1)
                av_fin(1, 0)
            if m == 6:
                av_mms(1, 1, range(SC))
            if m == 7:
                outproj_half1(0)
                outproj_half1(1)
        av_fin(1, 1)
        outproj_half1(2)
        outproj_half1(3)
        # window 3: scores(3) | attn_v(2) | rbc(2) | attn_v(3) rides the
        # exp stream at lag 1 chunk
        alloc_pair(3)
        for m in range(SC):
            scores_slot(3, m)
            if m == 1:
                outproj_half1(4)
                outproj_half1(5)
            if m == 2:
                av_mms(2, 0, range(SC))
            if m == 3:
                outproj_half1(6)
                outproj_half1(7)
            if m == 4:
                rbc_build(2)
                av_fin(2, 0)
            if m == 5:
                av_mms(2, 1, range(SC))
            if m == 6:
                av_fin(2, 1)
                outproj_mid2(0)
                outproj_mid2(1)
            if m == 7:
                outproj_mid2(2)
                outproj_mid2(3)
        for m in range(4, SC):
            outproj_mid2(m)
        av_mms(3, 0, range(SC))
        av_mms(3, 1, range(SC))
        # tail: pipelined rbc(3) + chunk-3 projections
        rbc_build3_split()

    nc.compile()
    return nc


def _get_nc():
    global _CACHED_NC
    if _CACHED_NC is None:
        _CACHED_NC = _build_nc()
    return _CACHED_NC


def _make_in_maps(inputs):
    query = np.asarray(inputs["query"], np.float32)
    value = np.asarray(inputs["value"], np.float32)
    import ml_dtypes
    bf = ml_dtypes.bfloat16
    w1 = np.asarray(inputs["w1"], np.float32)
    b1 = np.asarray(inputs["b1"], np.float32)
    w2 = np.asarray(inputs["w2"], np.float32)
    b2 = np.asarray(inputs["b2"], np.float32)
    wv = np.asarray(inputs["wv"], np.float32)
    bv = np.asarray(inputs["bv"], np.float32)
    wo = np.asarray(inputs["wo"], np.float32)
    bo = np.asarray(inputs["bo"], np.float32)

    b1r = np.ascontiguousarray(b1.reshape(HID // P, P).T)
    b2r = np.ascontiguousarray(b2.reshape(F // P, P).T)
    # softmax rows sum to 1, so the value bias commutes past attention:
    # out = attn(v @ wv) @ wo + (bv @ wo + bo)
    bo2 = bo + bv @ wo
    bo2b = np.ascontiguousarray(np.broadcast_to(bo2, (P, F)).astype(np.float32))

    shared = dict(w1=w1.astype(bf), w2=w2.astype(bf), wv=wv.astype(bf),
                  wo=wo.astype(bf), b1r=b1r, b2r=b2r, bo2b=bo2b,
                  ident=np.eye(P, dtype=np.float32))
    return [dict(qT=np.ascontiguousarray(query[i].T).astype(bf),
                 vT=np.ascontiguousarray(value[i].T).astype(bf), **shared)
            for i in range(N_CORES)]


def kernel(**inputs):
    in_maps = _make_in_maps(inputs)

    from concourse.bass_utils import run_bass_kernel_spmd

    nc = _get_nc()
    res = run_bass_kernel_spmd(nc, in_maps, core_ids=list(range(N_CORES)))
    out = np.stack([res.results[i]["out"] for i in range(N_CORES)], axis=0)
    return out.astype(np.float32)


if __name__ == "__main__":
    nc = _get_nc()
    print("built ok")


# revision 20
# speedup vs baseline: 1.0331x; 1.0331x over previous
"""Dense Synthesizer Attention — Trainium2 Bass kernel (v3).

Sharding: data-parallel over batch. B=8 batch elements, 8 NeuronCores,
one batch element per core, zero collectives.

Per-core (S=1024, F=512, H=8 heads, dk=64), bf16 matmuls, fp32 PSUM:
    hT  = relu(w1^T @ qT + b1)          [1024, 1024]
    awT = w2^T @ hT + b2                [512, 1024]
    per head pair: scores via ROW-TILED concurrent A/B matmuls
    (explicit tile_position (0,0)/(64,0)), E = exp(scores/8) on ScalarE,
    rowsums on DVE tensor_reduce (bf16 2x), attn_v via COL-TILED
    concurrent A/B matmuls, rbc = broadcast 1/rowsum via PE transposes,
    yT evac multiplies by rbc, then single-pass out projection.

Schedule: the exp stream on ScalarE is the pacer (64 x ~1.15us) and must
run back-to-back. Same-tiling-mode matmuls are grouped into bursts (mode
switches cost ~100ns). The scores PSUM ring is 2 deep so semaphore waits
resolve ahead of the PE (v2's single-buffered psA/psB serialized the A/B
pair at ACT cadence). PSUM banks: scores ring 2x[128,1024] (4) + pinned
[128,1024] (2: mlp2 chunks, then attn_v(3) accumulation; rbc transposes
go through the ring) + 2x[128,512] (vproj/attn_v/outproj).
"""

import math

import numpy as np

B, S, F = 8, 1024, 512
H, DK = 8, 64
HID = 2 * F
P = 128

N_CORES = 8

# pairs whose rowsums come from ACT accum_out instead of DVE tensor_reduce
ACT_ROWSUM_PAIRS = (3,)

_CACHED_NC = None


def _build_nc():
    from contextlib import ExitStack

    import concourse.mybir as mybir
    import concourse.tile as tile
    from concourse import bacc

    dt = mybir.dt
    f32 = dt.float32
    bf16 = dt.bfloat16
    AF = mybir.ActivationFunctionType
    ALU = mybir.AluOpType
    AX = mybir.AxisListType

    SC = S // P      # 8 token chunks
    FC = F // P      # 4 feature chunks
    KC = HID // P    # 8 hidden chunks
    NS = S // 512    # 2 moving-dim chunks
    NH = H // 2      # 4 head pairs

    nc = bacc.Bacc(
        "TRN2",
        target_bir_lowering=False,
        debug=False,
        num_devices=N_CORES,
    )

    q_d = nc.declare_dram_parameter("qT", [F, S], bf16, isOutput=False)
    v_d = nc.declare_dram_parameter("vT", [F, S], bf16, isOutput=False)
    w1_d = nc.declare_dram_parameter("w1", [F, HID], bf16, isOutput=False)
    w2_d = nc.declare_dram_parameter("w2", [HID, F], bf16, isOutput=False)
    wv_d = nc.declare_dram_parameter("wv", [F, F], bf16, isOutput=False)
    wo_d = nc.declare_dram_parameter("wo", [F, F], bf16, isOutput=False)
    b1_d = nc.declare_dram_parameter("b1r", [P, KC], f32, isOutput=False)
    b2_d = nc.declare_dram_parameter("b2r", [P, FC], f32, isOutput=False)
    bo_d = nc.declare_dram_parameter("bo2b", [P, F], f32, isOutput=False)
    id_d = nc.declare_dram_parameter("ident", [P, P], f32, isOutput=False)
    out_d = nc.declare_dram_parameter("out", [S, F], f32, isOutput=True)

    scale = 1.0 / math.sqrt(DK)

    with ExitStack() as ctx:
        tc = ctx.enter_context(tile.TileContext(nc))

        const = ctx.enter_context(tc.tile_pool(name="const", bufs=1))
        big = ctx.enter_context(tc.tile_pool(name="big", bufs=1))
        epool = ctx.enter_context(tc.tile_pool(name="ep", bufs=4))
        rpool = ctx.enter_context(tc.tile_pool(name="rp", bufs=2))
        opool = ctx.enter_context(tc.tile_pool(name="op", bufs=4))
        rbcpool = ctx.enter_context(tc.tile_pool(name="rbc", bufs=2))
        reppool = ctx.enter_context(tc.tile_pool(name="rep", bufs=2))

        ps_ring = ctx.enter_context(tc.tile_pool(name="psr", bufs=2, space="PSUM"))
        ps_pin = ctx.enter_context(tc.tile_pool(name="psp", bufs=1, space="PSUM"))
        ps_sml = ctx.enter_context(tc.tile_pool(name="pss", bufs=2, space="PSUM"))

        # ---- constants ----
        w1r = w1_d.rearrange("(c p) k -> p c k", p=P)
        w1sb = []
        w1_eng = [nc.gpsimd, nc.gpsimd, nc.scalar, None]
        for c in range(FC):
            t = const.tile([P, HID], bf16, name=f"w1c{c}")
            if w1_eng[c] is not None:
                w1_eng[c].dma_start(t, w1r[:, c, :])
            w1sb.append(t)
        b1sb = const.tile([P, KC], f32)
        nc.gpsimd.dma_start(b1sb, b1_d[:, :])
        w2sb = const.tile([P, KC, F], bf16)
        wvsb = const.tile([P, FC, F], bf16)
        wosb = const.tile([P, FC, F], bf16)
        b2sb = const.tile([P, FC], f32)
        bosb = const.tile([P, F], f32)
        identsb = const.tile([P, P], f32)

        def mid_consts():
            nc.gpsimd.dma_start(w2sb, w2_d.rearrange("(c p) f -> p c f", p=P))
            nc.gpsimd.dma_start(b2sb, b2_d[:, :])

        def late_consts():
            nc.gpsimd.dma_start(wvsb, wv_d.rearrange("(c p) f -> p c f", p=P))
            nc.gpsimd.dma_start(identsb, id_d[:, :])

        def last_consts():
            nc.gpsimd.dma_start(wosb, wo_d.rearrange("(c p) f -> p c f", p=P))
            nc.gpsimd.dma_start(bosb, bo_d[:, :])

        # ---- inputs ----
        qTsb = big.tile([P, FC, S], bf16, tag="qx")
        qr = q_d.rearrange("(c p) s -> p c s", p=P)
        nc.sync.dma_start(qTsb[:, :, :512], qr[:, :, :512])
        nc.sync.dma_start(qTsb[:, :, 512:], qr[:, :, 512:])
        nc.sync.dma_start(w1sb[3], w1r[:, 3, :])
        valTsb = big.tile([P, FC, S], bf16, tag="vT")
        nc.sync.dma_start(valTsb, v_d.rearrange("(c p) s -> p c s", p=P))
        mid_consts()

        hTsb = big.tile([P, KC, S], bf16, tag="hT")
        awTsb = big.tile([P, FC, S], bf16, tag="awT")
        vsb = big.tile([P, SC, F], bf16, tag="v")
        yTsb = big.tile([P, FC, S], bf16, tag="qx")  # reuses qT slot

        e_tiles = {}
        rsums = [None] * NH
        rinvs = [None] * NH
        rbcs = [None] * NH

        # ---- HAM warm-up: memset-seeded (no DMA dependency), one
        #      accumulation group (no inter-MM semaphores, gapless) ----
        warm_sb = const.tile([P, 512], bf16, name="warmsb")
        nc.vector.memset(warm_sb, 0.0)
        wt = ps_ring.tile([P, S], f32, tag="ring", name="warm")
        NWARM = 6
        for i in range(NWARM):
            nc.tensor.matmul(wt[:, 0:512], warm_sb[:, 0:P], warm_sb[:, 0:512],
                             start=(i == 0), stop=(i == NWARM - 1))

        # ---- mlp1 (full mode); mlp2(fc0) partials ride along in the
        #      pinned PSUM tile as hT chunks complete ----
        m2pin = {}

        def mlp2_fc0_fill(c):
            if c == 0:
                m2pin[0] = ps_pin.tile([P, S], f32, tag="pin", name="m2f")
            for n in range(NS):
                nc.tensor.matmul(
                    m2pin[0][:, n * 512:(n + 1) * 512],
                    w2sb[:, c, 0:P],
                    hTsb[:, c, n * 512:(n + 1) * 512],
                    start=(c == 0),
                    stop=(c == KC - 1),
                )

        for k in range(KC):
            ps = ps_ring.tile([P, S], f32, tag="ring", name=f"m1k{k}")
            for n in range(NS):
                for c in range(FC):
                    nc.tensor.matmul(
                        ps[:, n * 512:(n + 1) * 512],
                        w1sb[c][:, k * P:(k + 1) * P],
                        qTsb[:, c, n * 512:(n + 1) * 512],
                        start=(c == 0),
                        stop=(c == FC - 1),
                    )
            if k == KC - 1:
                nc.scalar.activation(hTsb[:, k, :], ps, AF.Relu,
                                     bias=b1sb[:, k:k + 1])
            else:
                nc.vector.tensor_scalar(
                    hTsb[:, k, :], ps, b1sb[:, k:k + 1], 0.0, ALU.add, ALU.max,
                )
            if k >= 1:
                mlp2_fc0_fill(k - 1)
            if k == 1:
                late_consts()
            if k == 4:
                last_consts()
        mlp2_fc0_fill(KC - 1)
        nc.scalar.activation(awTsb[:, 0, :], m2pin[0], AF.Identity,
                             bias=b2sb[:, 0:1])

        # ---- mlp2 chunk fc as two half-bursts (pinned PSUM); each half
        #      fits in the PE slack of one exp slot ----
        _m2pin = {}

        def mlp2_burst(fc, n):
            if n == 0:
                _m2pin[fc] = ps_pin.tile([P, S], f32, tag="pin", name=f"m2p{fc}")
            pin = _m2pin[fc]
            for c in range(KC):
                nc.tensor.matmul(
                    pin[:, n * 512:(n + 1) * 512],
                    w2sb[:, c, fc * P:(fc + 1) * P],
                    hTsb[:, c, n * 512:(n + 1) * 512],
                    start=(c == 0),
                    stop=(c == KC - 1),
                )
            if n == NS - 1:
                nc.vector.tensor_scalar_add(
                    awTsb[:, fc, :], pin, b2sb[:, fc:fc + 1])

        # ---- v projection for token tile m (full mode) ----
        def vproj(m):
            t = ps_sml.tile([P, F], f32, tag="sml", name="vpp")
            for c in range(FC):
                nc.tensor.matmul(
                    t,
                    valTsb[:, c, m * P:(m + 1) * P],
                    wvsb[:, c, :],
                    start=(c == 0),
                    stop=(c == FC - 1),
                )
            nc.vector.tensor_copy(vsb[:, m, :], t)

        # ---- scores slot (row-tiled pair, ring PSUM) + exp + rowsums ----
        def scores_slot(q, m):
            eA, eB = e_tiles[(q, 0)], e_tiles[(q, 1)]
            a_l = awTsb[0:DK, q, m * P:(m + 1) * P]
            b_l = awTsb[DK:P, q, m * P:(m + 1) * P]
            psa = ps_ring.tile([P, S], f32, tag="ring", name="sA")
            psb = ps_ring.tile([P, S], f32, tag="ring", name="sB")
            for n in range(NS):
                nc.tensor.matmul(
                    psa[:, n * 512:(n + 1) * 512],
                    a_l,
                    awTsb[0:DK, q, n * 512:(n + 1) * 512],
                    start=True, stop=True, tile_position=(0, 0),
                )
                nc.tensor.matmul(
                    psb[:, n * 512:(n + 1) * 512],
                    b_l,
                    awTsb[DK:P, q, n * 512:(n + 1) * 512],
                    start=True, stop=True, tile_position=(64, 0),
                )
            if q in ACT_ROWSUM_PAIRS:
                nc.scalar.activation(eA[:, m, :], psa, AF.Exp, scale=scale,
                                     accum_out=rsums[q][:, 0, m:m + 1])
                nc.scalar.activation(eB[:, m, :], psb, AF.Exp, scale=scale,
                                     accum_out=rsums[q][:, 1, m:m + 1])
            else:
                # head A rowsum on ACT, head B on DVE (balances both queues)
                nc.scalar.activation(eA[:, m, :], psa, AF.Exp, scale=scale,
                                     accum_out=rsums[q][:, 0, m:m + 1])
                nc.scalar.activation(eB[:, m, :], psb, AF.Exp, scale=scale)
                nc.vector.tensor_reduce(
                    rsums[q][:, 1, m:m + 1], eB[:, m, :], AX.X, ALU.add)

        # ---- rbc for pair q: recip + rep (DVE), 8 PE transposes into a
        #      ring tile, bounce to bf16 SBUF ----
        def rbc_build(q):
            nc.vector.reciprocal(rinvs[q], rsums[q])
            rep = reppool.tile([P, SC, 2, DK], f32, name="rep")
            nc.vector.tensor_copy(
                rep,
                rinvs[q][:, :, :].rearrange("p h m -> p m h")
                .broadcast_to((P, SC, 2, DK)),
            )
            rp = ps_pin.tile([P, S], f32, tag="pin", name="rbcp")
            for m in range(SC):
                nc.tensor.transpose(
                    rp[:, m * P:(m + 1) * P],
                    rep[:, m, :, :],
                    identsb[:, :],
                )
            rbcs[q] = rbcpool.tile([P, S], bf16, tag="rbc", name="rbcs")
            nc.vector.tensor_copy(rbcs[q], rp)

        # ---- attn_v (col-tiled pair). q<3 accumulates in a small tile per
        #      n-half; q==3 accumulates both halves in the pinned tile. ----
        _av_ps = {}

        def av_mms(q, n, chunks):
            eA, eB = e_tiles[(q, 0)], e_tiles[(q, 1)]
            for c in chunks:
                if c == 0:
                    _av_ps[(q, n)] = ps_sml.tile(
                        [P, F], f32, tag="sml", name="avp")
                t = _av_ps[(q, n)]
                nc.tensor.matmul(
                    t[0:DK, :],
                    vsb[:, c, (2 * q) * DK:(2 * q + 1) * DK],
                    eA[:, c, n * 512:(n + 1) * 512],
                    start=(c == 0), stop=(c == SC - 1), tile_position=(0, 0),
                )
                nc.tensor.matmul(
                    t[DK:P, :],
                    vsb[:, c, (2 * q + 1) * DK:(2 * q + 2) * DK],
                    eB[:, c, n * 512:(n + 1) * 512],
                    start=(c == 0), stop=(c == SC - 1), tile_position=(0, 64),
                )

        def av_fin(q, n):
            nc.vector.tensor_mul(
                yTsb[:, q, n * 512:(n + 1) * 512],
                _av_ps[(q, n)],
                rbcs[q][:, n * 512:(n + 1) * 512],
            )

        # ---- out projection, split: half1 contracts yT chunks 0,1
        #      (pairs 0,1) into o1 early; half2 adds chunks 2,3 + bias ----
        o1_sb = big.tile([P, SC, F], f32, tag="o1")

        def outproj_half1(m):
            t = ps_sml.tile([P, F], f32, tag="sml", name="op1")
            for c in (0, 1):
                nc.tensor.matmul(
                    t,
                    yTsb[:, c, m * P:(m + 1) * P],
                    wosb[:, c, :],
                    start=(c == 0),
                    stop=(c == 1),
                )
            nc.vector.tensor_add(o1_sb[:, m, :], t, bosb)

        def outproj_half2(m):
            t = ps_sml.tile([P, F], f32, tag="sml", name="op2")
            for c in (2, 3):
                nc.tensor.matmul(
                    t,
                    yTsb[:, c, m * P:(m + 1) * P],
                    wosb[:, c, :],
                    start=(c == 2),
                    stop=(c == 3),
                )
            o_sb = opool.tile([P, F], f32, tag="o", name="osb")
            nc.vector.tensor_add(o_sb, t, o1_sb[:, m, :])
            eng = nc.sync if m % 2 == 0 else nc.scalar
            eng.dma_start(out_d[m * P:(m + 1) * P, :], o_sb)

        def outproj_mid2(m):
            # stage yT chunk 2 (pair 2) into o1 during window 3
            t = ps_sml.tile([P, F], f32, tag="sml", name="om2")
            nc.tensor.matmul(t, yTsb[:, 2, m * P:(m + 1) * P], wosb[:, 2, :],
                             start=True, stop=True)
            nc.vector.tensor_add(o1_sb[:, m, :], t, o1_sb[:, m, :])

        def outproj_c3(m):
            t = ps_sml.tile([P, F], f32, tag="sml", name="oc3")
            nc.tensor.matmul(t, yTsb[:, 3, m * P:(m + 1) * P], wosb[:, 3, :],
                             start=True, stop=True)
            o_sb = opool.tile([P, F], f32, tag="o", name="osb")
            nc.vector.tensor_add(o_sb, t, o1_sb[:, m, :])
            eng = nc.sync if m % 2 == 0 else nc.scalar
            eng.dma_start(out_d[m * P:(m + 1) * P, :], o_sb)

        def rbc_build3_split():
            # pair-3 rbc pipelined in halves through the pin tile
            q = 3
            nc.vector.reciprocal(rinvs[q], rsums[q])
            rep = reppool.tile([P, SC, 2, DK], f32, name="rep")
            nc.vector.tensor_copy(
                rep,
                rinvs[q][:, :, :].rearrange("p h m -> p m h")
                .broadcast_to((P, SC, 2, DK)),
            )
            rp = ps_pin.tile([P, S], f32, tag="pin", name="rbcp3")
            rbcs[q] = rbcpool.tile([P, S], bf16, tag="rbc", name="rbcs")
            for m in range(4):
                nc.tensor.transpose(rp[:, m * P:(m + 1) * P], rep[:, m, :, :],
                                    identsb[:, :])
            nc.scalar.copy(rbcs[q][:, 0:512], rp[:, 0:512])
            av_fin(3, 0)
            for m in range(4):
                outproj_c3(m)
            for m in range(4, SC):
                nc.tensor.transpose(rp[:, m * P:(m + 1) * P], rep[:, m, :, :],
                                    identsb[:, :])
            nc.scalar.copy(rbcs[q][:, 512:], rp[:, 512:])
            av_fin(3, 1)
            for m in range(4, SC):
                outproj_c3(m)

        def alloc_pair(q):
            e_tiles[(q, 0)] = epool.tile([P, SC, S], bf16, tag="e", name="eA")
            e_tiles[(q, 1)] = epool.tile([P, SC, S], bf16, tag="e", name="eB")
            rsums[q] = rpool.tile([P, 2, SC], f32, tag="rs", name="rs")
            rinvs[q] = rpool.tile([P, 2, SC], f32, tag="ri", name="ri")

        # =========== pipeline ===========
        # window 0: scores(0) | mlp2 fc1 | vproj
        alloc_pair(0)
        for m in range(SC):
            scores_slot(0, m)
            if m == 1:
                mlp2_burst(1, 0)
            if m == 2:
                mlp2_burst(1, 1)
            if m >= 4:
                vproj(2 * (m - 4))
                vproj(2 * (m - 4) + 1)
        # window 1: scores(1) | attn_v(0) | mlp2 fc2 | rbc(0)
        alloc_pair(1)
        for m in range(SC):
            scores_slot(1, m)
            if m == 1:
                mlp2_burst(2, 0)
            if m == 2:
                mlp2_burst(2, 1)
            if m == 3:
                av_mms(0, 0, range(SC))
            if m == 5:
                rbc_build(0)
                av_fin(0, 0)
            if m == 6:
                av_mms(0, 1, range(SC))
        av_fin(0, 1)
        # window 2: scores(2) | attn_v(1) | mlp2 fc3 | rbc(1)
        alloc_pair(2)
        for m in range(SC):
            scores_slot(2, m)
            if m == 1:
                mlp2_burst(3, 0)
            if m == 2:
                mlp2_burst(3, 1)
            if m == 3:
                av_mms(1, 0, range(SC))
            if m == 5:
                rbc_build(1)
                av_fin(1, 0)
            if m == 6:
                av_mms(1, 1, range(SC))
            if m == 7:
                outproj_half1(0)
                outproj_half1(1)
        av_fin(1, 1)
        outproj_half1(2)
        outproj_half1(3)
        # window 3: scores(3) | attn_v(2) | rbc(2) | attn_v(3) rides the
        # exp stream at lag 1 chunk
        alloc_pair(3)
        for m in range(SC):
            scores_slot(3, m)
            if m == 1:
                outproj_half1(4)
                outproj_half1(5)
            if m == 2:
                av_mms(2, 0, range(SC))
            if m == 3:
                outproj_half1(6)
                outproj_half1(7)
            if m == 4:
                rbc_build(2)
                av_fin(2, 0)
            if m == 5:
                av_mms(2, 1, range(SC))
            if m == 6:
                av_fin(2, 1)
                outproj_mid2(0)
                outproj_mid2(1)
            if m == 7:
                outproj_mid2(2)
                outproj_mid2(3)
        for m in range(4, SC):
            outproj_mid2(m)
        av_mms(3, 0, range(SC))
        av_mms(3, 1, range(SC))
        # tail: pipelined rbc(3) + chunk-3 projections
        rbc_build3_split()

    nc.compile()
    return nc


def _get_nc():
    global _CACHED_NC
    if _CACHED_NC is None:
        _CACHED_NC = _build_nc()
    return _CACHED_NC


def _make_in_maps(inputs):
    query = np.asarray(inputs["query"], np.float32)
    value = np.asarray(inputs["value"], np.float32)
    import ml_dtypes
    bf = ml_dtypes.bfloat16
    w1 = np.asarray(inputs["w1"], np.float32)
    b1 = np.asarray(inputs["b1"], np.float32)
    w2 = np.asarray(inputs["w2"], np.float32)
    b2 = np.asarray(inputs["b2"], np.float32)
    wv = np.asarray(inputs["wv"], np.float32)
    bv = np.asarray(inputs["bv"], np.float32)
    wo = np.asarray(inputs["wo"], np.float32)
    bo = np.asarray(inputs["bo"], np.float32)

    b1r = np.ascontiguousarray(b1.reshape(HID // P, P).T)
    b2r = np.ascontiguousarray(b2.reshape(F // P, P).T)
    # softmax rows sum to 1, so the value bias commutes past attention:
    # out = attn(v @ wv) @ wo + (bv @ wo + bo)
    bo2 = bo + bv @ wo
    bo2b = np.ascontiguousarray(np.broadcast_to(bo2, (P, F)).astype(np.float32))

    shared = dict(w1=w1.astype(bf), w2=w2.astype(bf), wv=wv.astype(bf),
                  wo=wo.astype(bf), b1r=b1r, b2r=b2r, bo2b=bo2b,
                  ident=np.eye(P, dtype=np.float32))
    return [dict(qT=np.ascontiguousarray(query[i].T).astype(bf),
                 vT=np.ascontiguousarray(value[i].T).astype(bf), **shared)
            for i in range(N_CORES)]


def kernel(**inputs):
    in_maps = _make_in_maps(inputs)

    from concourse.bass_utils import run_bass_kernel_spmd

    nc = _get_nc()
    res = run_bass_kernel_spmd(nc, in_maps, core_ids=list(range(N_CORES)))
    out = np.stack([res.results[i]["out"] for i in range(N_CORES)], axis=0)
    return out.astype(np.float32)


if __name__ == "__main__":
    nc = _get_nc()
    print("built ok")
